# revision 10
# baseline (speedup 1.0000x reference)
"""GCN (3-layer + linear head) Trainium2 Bass kernel, sharded over 8 NeuronCores.

Strategy (matches the vertex-partitioning sharding hint):
 - Nodes are sharded contiguously: core c owns nodes [c*12500, (c+1)*12500),
   padded to 12544 = 98 blocks of 128 rows.
 - Per GCN layer, each core transforms its local rows (y = dinv * (h @ W)),
   the y shards are AllGathered (boundary/halo exchange), and each core
   aggregates messages for its own targets with dma_gather (per-edge source
   row fetch) + one-hot matmuls on the tensor engine that realize the
   segment-sum in PSUM.
 - All floating-point math runs on device. The host only does integer
   index work: adding self-loops, degree counting (bincount), sorting edges
   by (target-block-group, source-quarter, target), padding chunk counts so
   all 8 cores share one SPMD instruction stream.
"""
import os
import sys

sys.path.insert(0, "/opt/trn_rl_repo")

import numpy as np

_NLAYERS = int(os.environ.get("GCN_NLAYERS", "3"))
_SKIP_AGG = bool(int(os.environ.get("GCN_SKIP_AGG", "0")))
_SKIP_FINAL = bool(int(os.environ.get("GCN_SKIP_FINAL", "0")))
_MAX_GROUPS = int(os.environ.get("GCN_MAX_GROUPS", "999"))

import concourse.bacc as bacc
import concourse.mybir as mybir
import concourse.tile as tile
from concourse import bass_utils
from concourse.library_config import mlp

# Problem constants (hardcoded per harness contract).
N_NODES = 100000
D = 128
D_LAB = 10
NCORES = 8
SHARD = 12500
SHARD_P = 12544           # padded to 98 * 128
B = SHARD_P // 128        # 98 blocks per core
G = 6                     # target blocks per aggregation group (psum banks)
NG = -(-B // G)           # 17 groups
QROWS = 32768             # int16 index range per gather quarter
NQ = -(-(NCORES * SHARD_P) // QROWS)  # 4 quarters over padded global rows

F32 = mybir.dt.float32
I16 = mybir.dt.int16
AF = mybir.ActivationFunctionType
ALU = mybir.AluOpType

PAD_TGT = 9999.0          # one-hot target value for padded slots (matches nothing)


def _preprocess(edge_index):
    """Host-side integer/index prep. Returns per-core arrays + shared structure."""
    src = np.asarray(edge_index[0], dtype=np.int64)
    tgt = np.asarray(edge_index[1], dtype=np.int64)
    loops = np.arange(N_NODES, dtype=np.int64)
    src = np.concatenate([src, loops])
    tgt = np.concatenate([tgt, loops])

    deg = np.bincount(tgt, minlength=N_NODES).astype(np.int64)  # includes self loop

    core = tgt // SHARD
    tl = tgt % SHARD                       # target local to core
    gs = (src // SHARD) * SHARD_P + (src % SHARD)  # padded global source row
    q = gs // QROWS
    qrel = (gs - q * QROWS).astype(np.int64)
    blk = tl // 128
    grp = blk // G

    order = np.lexsort((tl, q, grp, core))
    core_s, tl_s, q_s, qrel_s, blk_s = (
        core[order], tl[order], q[order], qrel[order], blk[order])

    # segment = (core, g, q, blk); count edges per segment
    seg_key = ((core_s * NG + (blk_s // G)) * NQ + q_s) * B + blk_s
    counts = np.bincount(seg_key, minlength=NCORES * NG * NQ * B).reshape(
        NCORES, NG, NQ, B)
    # structural chunk count per (g, q, blk): max over cores
    C = -(-counts // 128)
    C = C.max(axis=0)  # [NG, NQ, B]

    # Build flat idx / tgt arrays per core, in (g, q, b, chunk) order.
    tot_chunks = int(C.sum())
    TOT = tot_chunks * 128
    idx_all = np.zeros((NCORES, TOT), dtype=np.int16)
    tgt_all = np.full((NCORES, tot_chunks * 128), PAD_TGT, dtype=np.float32)

    # per-core start offset of each segment in the sorted arrays
    seg_starts = np.zeros(NCORES * NG * NQ * B + 1, dtype=np.int64)
    np.cumsum(np.bincount(seg_key, minlength=NCORES * NG * NQ * B),
              out=seg_starts[1:])

    # shared structure for the builder
    segs = []       # list of (g, q, b, n_chunks) in emission order
    nch_gq = np.zeros((NG, NQ), dtype=np.int64)
    off = 0
    for g in range(NG):
        for qq in range(NQ):
            for b in range(g * G, min((g + 1) * G, B)):
                nch = int(C[g, qq, b])
                if nch == 0:
                    continue
                segs.append((g, qq, b, nch, off))
                nch_gq[g, qq] += nch
                for c in range(NCORES):
                    k = ((c * NG + g) * NQ + qq) * B + b
                    s0, s1 = seg_starts[k], seg_starts[k + 1]
                    n = s1 - s0
                    idx_all[c, off:off + n] = qrel_s[s0:s1]
                    tgt_all[c, off:off + n] = (tl_s[s0:s1] - b * 128)
                off += nch * 128
    assert off == TOT

    # wrap idxs to [128, TOT/16]: idx i -> [i % 16, i // 16], tiled x8
    idx_wrapped = np.stack([
        np.tile(a.reshape(-1, 16).T, (8, 1)) for a in idx_all])
    # tgt values in [128, tot_chunks]: chunk k, slot p -> [p, k]
    tgt_tiles = tgt_all.reshape(NCORES, tot_chunks, 128).transpose(0, 2, 1).copy()

    # degrees, padded shards; pad deg with 1 to avoid inf (padded rows harmless)
    deg_p = np.ones((NCORES, SHARD_P), dtype=np.float32)
    deg_p[:, :SHARD] = deg.reshape(NCORES, SHARD).astype(np.float32)
    deg_col = deg_p.reshape(NCORES, B, 128).transpose(0, 2, 1).copy()  # [c,128,B]
    deg_row = deg_p.reshape(NCORES, 1, SHARD_P)

    return dict(idx=idx_wrapped, tgt=tgt_tiles, deg_col=deg_col, deg_row=deg_row,
                segs=segs, nch_gq=nch_gq, tot_chunks=tot_chunks, TOT=TOT)


def _build(pre):
    """Build the Bass/Tile program (one SPMD NEFF for all 8 cores)."""
    TOT = pre["TOT"]
    tot_chunks = pre["tot_chunks"]
    nch_gq = pre["nch_gq"]
    segs = pre["segs"]

    nc = bacc.Bacc("TRN2", target_bir_lowering=False, debug=False,
                   num_devices=NCORES)

    feat_d = nc.dram_tensor("feat", [SHARD_P, D], F32, kind="ExternalInput")
    idx_d = nc.dram_tensor("idx", [128, TOT // 16], I16, kind="ExternalInput")
    tgt_d = nc.dram_tensor("tgt", [128, tot_chunks], F32, kind="ExternalInput")
    degc_d = nc.dram_tensor("deg_col", [128, B], F32, kind="ExternalInput")
    degr_d = nc.dram_tensor("deg_row", [1, SHARD_P], F32, kind="ExternalInput")
    w_d = nc.dram_tensor("w_all", [128, 3 * D], F32, kind="ExternalInput")
    b_d = nc.dram_tensor("b_all", [1, 3 * D], F32, kind="ExternalInput")
    wp_d = nc.dram_tensor("wp_all", [128, 3 * D_LAB], F32, kind="ExternalInput")
    bp_d = nc.dram_tensor("bp", [1, D_LAB], F32, kind="ExternalInput")
    iota_d = nc.dram_tensor("iota", [128, 128], F32, kind="ExternalInput")
    ident_d = nc.dram_tensor("ident", [128, 128], F32, kind="ExternalInput")

    out_d = nc.dram_tensor("out", [D_LAB, SHARD_P], F32, kind="ExternalOutput")

    with tile.TileContext(nc) as tc:
        with (
            tc.tile_pool(name="const", bufs=1) as cpool,
            tc.tile_pool(name="work", bufs=3) as wpool,
            tc.tile_pool(name="sbuild", bufs=8) as spool,
            tc.tile_pool(name="mtiles", bufs=2) as mpool,
            tc.tile_pool(name="psum_a", bufs=G, space="PSUM") as ppa,
            tc.tile_pool(name="psum_t", bufs=2, space="PSUM") as ppt,
            tc.tile_pool(name="dram", bufs=1, space="DRAM") as dpool,
        ):
            nc.gpsimd.load_library(mlp)

            # ---- constants ----
            tgt_s = cpool.tile([128, tot_chunks], F32)
            iota_s = cpool.tile([128, 128], F32)
            ident_s = cpool.tile([128, 128], F32)
            w_s = cpool.tile([128, 3 * D], F32)
            b_s = cpool.tile([1, 3 * D], F32)
            wp_s = cpool.tile([128, 3 * D_LAB], F32)
            bp_s = cpool.tile([1, D_LAB], F32)
            ones_s = cpool.tile([1, 128], F32)
            degc_s = cpool.tile([128, B], F32)
            recip_s = cpool.tile([128, B], F32)
            dinv_s = cpool.tile([128, B], F32)
            sqdeg_s = cpool.tile([1, SHARD_P], F32)

            nc.sync.dma_start(tgt_s[:], tgt_d[:])
            nc.sync.dma_start(iota_s[:], iota_d[:])
            nc.sync.dma_start(ident_s[:], ident_d[:])
            nc.sync.dma_start(w_s[:], w_d[:])
            nc.sync.dma_start(b_s[:], b_d[:])
            nc.sync.dma_start(wp_s[:], wp_d[:])
            nc.sync.dma_start(bp_s[:], bp_d[:])
            nc.sync.dma_start(degc_s[:], degc_d[:])
            nc.sync.dma_start(sqdeg_s[:], degr_d[:])
            nc.vector.memset(ones_s[:], 1.0)

            # dinv = sqrt(1/deg)  (accurate DVE reciprocal + ACT sqrt)
            nc.vector.reciprocal(recip_s[:], degc_s[:])
            nc.scalar.sqrt(dinv_s[:], recip_s[:])
            nc.scalar.sqrt(sqdeg_s[:], sqdeg_s[:])  # in-place: deg -> sqrt(deg)

            # ---- internal DRAM ----
            y_loc = dpool.tile([SHARD_P, D], F32)
            y_fulls = [
                dpool.tile([NCORES * SHARD_P, D], F32, addr_space="Shared",
                           name=f"y_full_{i}")
                for i in range(3)
            ]
            h1 = dpool.tile([SHARD_P, D], F32)
            h2 = dpool.tile([SHARD_P, D], F32)
            h3 = dpool.tile([SHARD_P, D], F32)

            h_tensors = [feat_d, h1, h2, h3]

            for layer in range(_NLAYERS):
                hin = h_tensors[layer]
                hout = h_tensors[layer + 1]
                wl = w_s[:, layer * D:(layer + 1) * D]
                bl = b_s[:, layer * D:(layer + 1) * D]

                # -- transform: y = dinv * (hin @ W) --
                for b in range(B):
                    r0, r1 = b * 128, (b + 1) * 128
                    hb = wpool.tile([128, 128], F32, tag="hb")
                    nc.sync.dma_start(hb[:], hin[r0:r1, :])
                    tp = ppt.tile([128, 128], F32, tag="tp")
                    nc.tensor.transpose(tp[:], hb[:], ident_s[:])
                    hT = wpool.tile([128, 128], F32, tag="hT")
                    nc.vector.tensor_copy(hT[:], tp[:])
                    yp = ppt.tile([128, 128], F32, tag="tp")
                    nc.tensor.matmul(yp[:], hT[:], wl, start=True, stop=True)
                    yb = wpool.tile([128, 128], F32, tag="yb")
                    nc.scalar.activation(yb[:], yp[:], AF.Copy,
                                         bias=0.0, scale=dinv_s[:, b:b + 1])
                    nc.sync.dma_start(y_loc[r0:r1, :], yb[:])

                # -- halo exchange --
                y_full = y_fulls[layer]
                nc.gpsimd.collective_compute(
                    "AllGather", ALU.bypass,
                    replica_groups=[list(range(NCORES))],
                    ins=[y_loc.opt()], outs=[y_full.opt()],
                )

                # -- aggregate --
                if _SKIP_AGG:
                    continue
                seg_i = 0
                ci = 0          # global chunk counter (matches tgt_s columns)
                off16 = 0       # idx column offset
                for g in range(min(NG, _MAX_GROUPS)):
                    blocks = list(range(g * G, min((g + 1) * G, B)))
                    psums = {b: ppa.tile([128, 128], F32, tag="agg",
                                         name=f"ps_{layer}_{b}")
                             for b in blocks}
                    started = {b: False for b in blocks}
                    for qq in range(NQ):
                        nch = int(nch_gq[g, qq])
                        if nch == 0:
                            continue
                        n_idx = nch * 128
                        n16 = n_idx // 16
                        idx_t = mpool.tile([128, n16], I16, tag="idxs",
                                           name=f"ix_{layer}_{g}_{qq}")
                        nc.sync.dma_start(idx_t[:],
                                          idx_d[:, off16:off16 + n16])
                        mt = mpool.tile([128, nch, 128], F32, tag="m",
                                        name=f"m_{layer}_{g}_{qq}")
                        qs = qq * QROWS
                        qe = min(qs + QROWS, NCORES * SHARD_P)
                        nc.gpsimd.dma_gather(
                            mt[:], y_full[qs:qe, :], idx_t[:],
                            n_idx, n_idx, D, single_packet=False)
                        off16 += n16
                        k = 0
                        while k < nch:
                            g2, q2, b2, nck, _ = segs[seg_i]
                            assert g2 == g and q2 == qq
                            for _ in range(nck):
                                st = spool.tile([128, 128], F32, tag="s",
                                                name=f"s_{layer}_{ci}")
                                nc.vector.tensor_scalar(
                                    st[:], iota_s[:],
                                    tgt_s[:, ci:ci + 1], None,
                                    ALU.is_equal)
                                nc.tensor.matmul(
                                    psums[b2][:], st[:], mt[:, k, :],
                                    start=not started[b2], stop=False)
                                started[b2] = True
                                k += 1
                                ci += 1
                            seg_i += 1
                    for b in blocks:
                        assert started[b]
                        r0, r1 = b * 128, (b + 1) * 128
                        # psum += sqrt(deg) x bias  (rank-1)
                        nc.tensor.matmul(psums[b][:], sqdeg_s[:, r0:r1], bl,
                                         start=False, stop=True)
                        ob = wpool.tile([128, 128], F32, tag="ob")
                        func = AF.Relu if layer < 2 else AF.Copy
                        nc.scalar.activation(ob[:], psums[b][:], func,
                                             bias=0.0, scale=dinv_s[:, b:b + 1])
                        nc.sync.dma_start(hout[r0:r1, :], ob[:])
                if _MAX_GROUPS >= NG:
                    assert seg_i == len(segs) and ci == tot_chunks

            # -- final projection: out[l, t] = sum_i h_i @ Wp_i + bp --
            for b in range(B if not (_SKIP_FINAL or _NLAYERS < 3) else 0):
                r0, r1 = b * 128, (b + 1) * 128
                pf = ppt.tile([D_LAB, 128], F32, tag="tp", name=f"pf_{b}")
                for i, hd in enumerate((h1, h2, h3)):
                    fb = wpool.tile([128, 128], F32, tag="hb")
                    nc.sync.dma_start(fb[:], hd[r0:r1, :])
                    ftp = ppt.tile([128, 128], F32, tag="tp")
                    nc.tensor.transpose(ftp[:], fb[:], ident_s[:])
                    fT = wpool.tile([128, 128], F32, tag="hT")
                    nc.vector.tensor_copy(fT[:], ftp[:])
                    nc.tensor.matmul(pf[:], wp_s[:, i * D_LAB:(i + 1) * D_LAB],
                                     fT[:], start=(i == 0), stop=False)
                nc.tensor.matmul(pf[:], bp_s[:], ones_s[:],
                                 start=False, stop=True)
                fo = wpool.tile([D_LAB, 128], F32, tag="fo")
                nc.scalar.activation(fo[:], pf[:], AF.Copy)
                nc.sync.dma_start(out_d[:, r0:r1], fo[:])

    nc.compile()
    return nc


_CACHE = {}


def _get_program(edge_index):
    key = hash(np.asarray(edge_index).tobytes())
    if key not in _CACHE:
        pre = _preprocess(edge_index)
        nc = _build(pre)
        _CACHE.clear()
        _CACHE[key] = (pre, nc)
    return _CACHE[key]


def kernel(feat, edge_index, W1, b1, W2, b2, W3, b3, Wp, bp):
    feat = np.asarray(feat, np.float32)
    edge_index = np.asarray(edge_index, np.int32)
    W1, b1, W2, b2, W3, b3, Wp, bp = (np.asarray(a, np.float32)
                                      for a in (W1, b1, W2, b2, W3, b3, Wp, bp))
    pre, nc = _get_program(edge_index)

    w_all = np.concatenate([W1, W2, W3], axis=1)              # [128, 384]
    b_all = np.concatenate([b1, b2, b3]).reshape(1, 3 * D)
    wp_all = np.concatenate([Wp[:D], Wp[D:2 * D], Wp[2 * D:]], axis=1)  # [128,30]
    iota = np.broadcast_to(np.arange(128, dtype=np.float32), (128, 128)).copy()
    ident = np.eye(128, dtype=np.float32)

    feat_p = np.zeros((NCORES, SHARD_P, D), np.float32)
    feat_p[:, :SHARD] = feat.reshape(NCORES, SHARD, D)

    in_maps = []
    for c in range(NCORES):
        in_maps.append({
            "feat": feat_p[c],
            "idx": pre["idx"][c],
            "tgt": pre["tgt"][c],
            "deg_col": pre["deg_col"][c],
            "deg_row": pre["deg_row"][c],
            "w_all": w_all, "b_all": b_all,
            "wp_all": wp_all, "bp": bp.reshape(1, D_LAB),
            "iota": iota, "ident": ident,
        })

    res = bass_utils.run_bass_kernel_spmd(nc, in_maps,
                                          core_ids=list(range(NCORES)))
    out = np.empty((N_NODES, D_LAB), np.float32)
    for c in range(NCORES):
        out[c * SHARD:(c + 1) * SHARD] = res.results[c]["out"].T[:SHARD]
    return out


# revision 13
# speedup vs baseline: 199.3511x; 199.3511x over previous
"""GCN (3-layer + linear head) Trainium2 Bass kernel, sharded over 8 NeuronCores.

Strategy (matches the vertex-partitioning sharding hint):
 - Nodes are sharded contiguously: core c owns nodes [c*12500, (c+1)*12500),
   padded to 12544 = 98 blocks of 128 rows.
 - Per GCN layer, each core transforms its local rows (y = dinv * (h @ W)),
   the y shards are AllGathered (boundary/halo exchange), and each core
   aggregates messages for its own targets with dma_gather (per-edge source
   row fetch) + one-hot matmuls on the tensor engine that realize the
   segment-sum in PSUM.
 - All floating-point math runs on device. The host only does integer
   index work: adding self-loops, degree counting (bincount), sorting edges
   by (target-block-group, source-quarter, target), padding chunk counts so
   all 8 cores share one SPMD instruction stream.
"""
import os
import sys

sys.path.insert(0, "/opt/trn_rl_repo")

import numpy as np

_NLAYERS = int(os.environ.get("GCN_NLAYERS", "3"))
_SKIP_AGG = bool(int(os.environ.get("GCN_SKIP_AGG", "0")))
_SKIP_FINAL = bool(int(os.environ.get("GCN_SKIP_FINAL", "0")))
_MAX_GROUPS = int(os.environ.get("GCN_MAX_GROUPS", "999"))

import concourse.bacc as bacc
import concourse.mybir as mybir
import concourse.tile as tile
from concourse import bass_utils
from concourse.library_config import mlp

# Problem constants (hardcoded per harness contract).
N_NODES = 100000
D = 128
D_LAB = 10
NCORES = 8
SHARD = 12500
SHARD_P = 12544           # padded to 98 * 128
B = SHARD_P // 128        # 98 blocks per core
G = 6                     # target blocks per aggregation group (psum banks)
NG = -(-B // G)           # 17 groups
QROWS = 32768             # int16 index range per gather quarter
NQ = -(-(NCORES * SHARD_P) // QROWS)  # 4 quarters over padded global rows

F32 = mybir.dt.float32
I16 = mybir.dt.int16
AF = mybir.ActivationFunctionType
ALU = mybir.AluOpType

PAD_TGT = 9999.0          # one-hot target value for padded slots (matches nothing)


def _preprocess(edge_index):
    """Host-side integer/index prep. Returns per-core arrays + shared structure."""
    src = np.asarray(edge_index[0], dtype=np.int64)
    tgt = np.asarray(edge_index[1], dtype=np.int64)
    loops = np.arange(N_NODES, dtype=np.int64)
    src = np.concatenate([src, loops])
    tgt = np.concatenate([tgt, loops])

    deg = np.bincount(tgt, minlength=N_NODES).astype(np.int64)  # includes self loop

    core = tgt // SHARD
    tl = tgt % SHARD                       # target local to core
    gs = (src // SHARD) * SHARD_P + (src % SHARD)  # padded global source row
    q = gs // QROWS
    qrel = (gs - q * QROWS).astype(np.int64)
    blk = tl // 128
    grp = blk // G

    order = np.lexsort((tl, q, grp, core))
    core_s, tl_s, q_s, qrel_s, blk_s = (
        core[order], tl[order], q[order], qrel[order], blk[order])

    # segment = (core, g, q, blk); count edges per segment
    seg_key = ((core_s * NG + (blk_s // G)) * NQ + q_s) * B + blk_s
    counts = np.bincount(seg_key, minlength=NCORES * NG * NQ * B).reshape(
        NCORES, NG, NQ, B)
    # structural chunk count per (g, q, blk): max over cores
    C = -(-counts // 128)
    C = C.max(axis=0)  # [NG, NQ, B]

    # Build flat idx / tgt arrays per core, in (g, q, b, chunk) order.
    tot_chunks = int(C.sum())
    TOT = tot_chunks * 128
    idx_all = np.zeros((NCORES, TOT), dtype=np.int16)
    tgt_all = np.full((NCORES, tot_chunks * 128), PAD_TGT, dtype=np.float32)

    # per-core start offset of each segment in the sorted arrays
    seg_starts = np.zeros(NCORES * NG * NQ * B + 1, dtype=np.int64)
    np.cumsum(np.bincount(seg_key, minlength=NCORES * NG * NQ * B),
              out=seg_starts[1:])

    # shared structure for the builder
    segs = []       # list of (g, q, b, n_chunks) in emission order
    nch_gq = np.zeros((NG, NQ), dtype=np.int64)
    off = 0
    for g in range(NG):
        for qq in range(NQ):
            for b in range(g * G, min((g + 1) * G, B)):
                nch = int(C[g, qq, b])
                if nch == 0:
                    continue
                segs.append((g, qq, b, nch, off))
                nch_gq[g, qq] += nch
                for c in range(NCORES):
                    k = ((c * NG + g) * NQ + qq) * B + b
                    s0, s1 = seg_starts[k], seg_starts[k + 1]
                    n = s1 - s0
                    idx_all[c, off:off + n] = qrel_s[s0:s1]
                    tgt_all[c, off:off + n] = (tl_s[s0:s1] - b * 128)
                off += nch * 128
    assert off == TOT

    # wrap idxs to [128, TOT/16]: idx i -> [i % 16, i // 16], tiled x8
    idx_wrapped = np.stack([
        np.tile(a.reshape(-1, 16).T, (8, 1)) for a in idx_all])
    # tgt values in [128, tot_chunks]: chunk k, slot p -> [p, k]
    tgt_tiles = tgt_all.reshape(NCORES, tot_chunks, 128).transpose(0, 2, 1).copy()

    # degrees, padded shards; pad deg with 1 to avoid inf (padded rows harmless)
    deg_p = np.ones((NCORES, SHARD_P), dtype=np.float32)
    deg_p[:, :SHARD] = deg.reshape(NCORES, SHARD).astype(np.float32)
    deg_col = deg_p.reshape(NCORES, B, 128).transpose(0, 2, 1).copy()  # [c,128,B]
    deg_row = deg_p.reshape(NCORES, 1, SHARD_P)

    return dict(idx=idx_wrapped, tgt=tgt_tiles, deg_col=deg_col, deg_row=deg_row,
                segs=segs, nch_gq=nch_gq, tot_chunks=tot_chunks, TOT=TOT)


def _build(pre):
    """Build the Bass/Tile program (one SPMD NEFF for all 8 cores)."""
    TOT = pre["TOT"]
    tot_chunks = pre["tot_chunks"]
    nch_gq = pre["nch_gq"]
    segs = pre["segs"]

    nc = bacc.Bacc("TRN2", target_bir_lowering=False, debug=False,
                   num_devices=NCORES)

    feat_d = nc.dram_tensor("feat", [SHARD_P, D], F32, kind="ExternalInput")
    idx_d = nc.dram_tensor("idx", [128, TOT // 16], I16, kind="ExternalInput")
    tgt_d = nc.dram_tensor("tgt", [128, tot_chunks], F32, kind="ExternalInput")
    degc_d = nc.dram_tensor("deg_col", [128, B], F32, kind="ExternalInput")
    degr_d = nc.dram_tensor("deg_row", [1, SHARD_P], F32, kind="ExternalInput")
    w_d = nc.dram_tensor("w_all", [128, 3 * D], F32, kind="ExternalInput")
    b_d = nc.dram_tensor("b_all", [1, 3 * D], F32, kind="ExternalInput")
    wp_d = nc.dram_tensor("wp_all", [128, 3 * D_LAB], F32, kind="ExternalInput")
    bp_d = nc.dram_tensor("bp", [1, D_LAB], F32, kind="ExternalInput")
    iota_d = nc.dram_tensor("iota", [128, 128], F32, kind="ExternalInput")
    ident_d = nc.dram_tensor("ident", [128, 128], F32, kind="ExternalInput")

    out_d = nc.dram_tensor("out", [D_LAB, SHARD_P], F32, kind="ExternalOutput")

    with tile.TileContext(nc) as tc:
        with (
            tc.tile_pool(name="const", bufs=1) as cpool,
            tc.tile_pool(name="work", bufs=3) as wpool,
            tc.tile_pool(name="sbuild", bufs=8) as spool,
            tc.tile_pool(name="mtiles", bufs=2) as mpool,
            tc.tile_pool(name="psum_a", bufs=G, space="PSUM") as ppa,
            tc.tile_pool(name="psum_t", bufs=2, space="PSUM") as ppt,
            tc.tile_pool(name="dram", bufs=1, space="DRAM") as dpool,
        ):
            nc.gpsimd.load_library(mlp)

            # ---- constants ----
            tgt_s = cpool.tile([128, tot_chunks], F32)
            iota_s = cpool.tile([128, 128], F32)
            ident_s = cpool.tile([128, 128], F32)
            w_s = cpool.tile([128, 3 * D], F32)
            b_s = cpool.tile([1, 3 * D], F32)
            wp_s = cpool.tile([128, 3 * D_LAB], F32)
            bp_s = cpool.tile([1, D_LAB], F32)
            ones_s = cpool.tile([1, 128], F32)
            degc_s = cpool.tile([128, B], F32)
            recip_s = cpool.tile([128, B], F32)
            dinv_s = cpool.tile([128, B], F32)
            sqdeg_s = cpool.tile([1, SHARD_P], F32)

            nc.sync.dma_start(tgt_s[:], tgt_d[:])
            nc.sync.dma_start(iota_s[:], iota_d[:])
            nc.sync.dma_start(ident_s[:], ident_d[:])
            nc.sync.dma_start(w_s[:], w_d[:])
            nc.sync.dma_start(b_s[:], b_d[:])
            nc.sync.dma_start(wp_s[:], wp_d[:])
            nc.sync.dma_start(bp_s[:], bp_d[:])
            nc.sync.dma_start(degc_s[:], degc_d[:])
            nc.sync.dma_start(sqdeg_s[:], degr_d[:])
            nc.vector.memset(ones_s[:], 1.0)

            # dinv = sqrt(1/deg)  (accurate DVE reciprocal + ACT sqrt)
            nc.vector.reciprocal(recip_s[:], degc_s[:])
            nc.scalar.sqrt(dinv_s[:], recip_s[:])
            nc.scalar.sqrt(sqdeg_s[:], sqdeg_s[:])  # in-place: deg -> sqrt(deg)

            # ---- internal DRAM ----
            y_loc = dpool.tile([SHARD_P, D], F32)
            y_fulls = [
                dpool.tile([NCORES * SHARD_P, D], F32, addr_space="Shared",
                           name=f"y_full_{i}")
                for i in range(3)
            ]
            h1 = dpool.tile([SHARD_P, D], F32)
            h2 = dpool.tile([SHARD_P, D], F32)
            h3 = dpool.tile([SHARD_P, D], F32)

            h_tensors = [feat_d, h1, h2, h3]

            for layer in range(_NLAYERS):
                hin = h_tensors[layer]
                hout = h_tensors[layer + 1]
                wl = w_s[:, layer * D:(layer + 1) * D]
                bl = b_s[:, layer * D:(layer + 1) * D]

                # -- transform: y = dinv * (hin @ W) --
                for b in range(B):
                    r0, r1 = b * 128, (b + 1) * 128
                    hb = wpool.tile([128, 128], F32, tag="hb")
                    nc.sync.dma_start(hb[:], hin[r0:r1, :])
                    tp = ppt.tile([128, 128], F32, tag="tp")
                    nc.tensor.transpose(tp[:], hb[:], ident_s[:])
                    hT = wpool.tile([128, 128], F32, tag="hT")
                    nc.vector.tensor_copy(hT[:], tp[:])
                    yp = ppt.tile([128, 128], F32, tag="tp")
                    nc.tensor.matmul(yp[:], hT[:], wl, start=True, stop=True)
                    yb = wpool.tile([128, 128], F32, tag="yb")
                    nc.scalar.activation(yb[:], yp[:], AF.Copy,
                                         bias=0.0, scale=dinv_s[:, b:b + 1])
                    nc.sync.dma_start(y_loc[r0:r1, :], yb[:])

                # -- halo exchange --
                y_full = y_fulls[layer]
                nc.gpsimd.collective_compute(
                    "AllGather", ALU.bypass,
                    replica_groups=[list(range(NCORES))],
                    ins=[y_loc.opt()], outs=[y_full.opt()],
                )

                # -- aggregate --
                if _SKIP_AGG:
                    continue
                seg_i = 0
                ci = 0          # global chunk counter (matches tgt_s columns)
                off16 = 0       # idx column offset
                for g in range(min(NG, _MAX_GROUPS)):
                    blocks = list(range(g * G, min((g + 1) * G, B)))
                    psums = {b: ppa.tile([128, 128], F32, tag="agg",
                                         name=f"ps_{layer}_{b}")
                             for b in blocks}
                    started = {b: False for b in blocks}
                    for qq in range(NQ):
                        nch = int(nch_gq[g, qq])
                        if nch == 0:
                            continue
                        n_idx = nch * 128
                        n16 = n_idx // 16
                        idx_t = mpool.tile([128, n16], I16, tag="idxs",
                                           name=f"ix_{layer}_{g}_{qq}")
                        nc.sync.dma_start(idx_t[:],
                                          idx_d[:, off16:off16 + n16])
                        mt = mpool.tile([128, nch, 128], F32, tag="m",
                                        name=f"m_{layer}_{g}_{qq}")
                        qs = qq * QROWS
                        qe = min(qs + QROWS, NCORES * SHARD_P)
                        nc.gpsimd.dma_gather(
                            mt[:], y_full[qs:qe, :], idx_t[:],
                            n_idx, n_idx, D, single_packet=False)
                        off16 += n16
                        k = 0
                        while k < nch:
                            g2, q2, b2, nck, _ = segs[seg_i]
                            assert g2 == g and q2 == qq
                            for _ in range(nck):
                                st = spool.tile([128, 128], F32, tag="s",
                                                name=f"s_{layer}_{ci}")
                                nc.vector.tensor_scalar(
                                    st[:], iota_s[:],
                                    tgt_s[:, ci:ci + 1], None,
                                    ALU.is_equal)
                                nc.tensor.matmul(
                                    psums[b2][:], st[:], mt[:, k, :],
                                    start=not started[b2], stop=False)
                                started[b2] = True
                                k += 1
                                ci += 1
                            seg_i += 1
                    for b in blocks:
                        assert started[b]
                        r0, r1 = b * 128, (b + 1) * 128
                        # psum += sqrt(deg) x bias  (rank-1)
                        nc.tensor.matmul(psums[b][:], sqdeg_s[:, r0:r1], bl,
                                         start=False, stop=True)
                        ob = wpool.tile([128, 128], F32, tag="ob")
                        func = AF.Relu if layer < 2 else AF.Copy
                        nc.scalar.activation(ob[:], psums[b][:], func,
                                             bias=0.0, scale=dinv_s[:, b:b + 1])
                        nc.sync.dma_start(hout[r0:r1, :], ob[:])
                if _MAX_GROUPS >= NG:
                    assert seg_i == len(segs) and ci == tot_chunks

            # -- final projection: out[l, t] = sum_i h_i @ Wp_i + bp --
            for b in range(B if not (_SKIP_FINAL or _NLAYERS < 3) else 0):
                r0, r1 = b * 128, (b + 1) * 128
                pf = ppt.tile([D_LAB, 128], F32, tag="tp", name=f"pf_{b}")
                for i, hd in enumerate((h1, h2, h3)):
                    fb = wpool.tile([128, 128], F32, tag="hb")
                    nc.sync.dma_start(fb[:], hd[r0:r1, :])
                    ftp = ppt.tile([128, 128], F32, tag="tp")
                    nc.tensor.transpose(ftp[:], fb[:], ident_s[:])
                    fT = wpool.tile([128, 128], F32, tag="hT")
                    nc.vector.tensor_copy(fT[:], ftp[:])
                    nc.tensor.matmul(pf[:], wp_s[:, i * D_LAB:(i + 1) * D_LAB],
                                     fT[:], start=(i == 0), stop=False)
                nc.tensor.matmul(pf[:], bp_s[:], ones_s[:],
                                 start=False, stop=True)
                fo = wpool.tile([D_LAB, 128], F32, tag="fo")
                nc.scalar.activation(fo[:], pf[:], AF.Copy)
                nc.sync.dma_start(out_d[:, r0:r1], fo[:])

    nc.compile()
    return nc


_CACHE = {}


def _get_program(edge_index):
    key = hash(np.asarray(edge_index).tobytes())
    if key not in _CACHE:
        pre = _preprocess(edge_index)
        nc = _build(pre)
        _CACHE.clear()
        _CACHE[key] = (pre, nc)
    return _CACHE[key]


def prepare(feat, edge_index, W1, b1, W2, b2, W3, b3, Wp, bp):
    """Build (nc, in_maps) for the SPMD run."""
    feat = np.asarray(feat, np.float32)
    edge_index = np.asarray(edge_index, np.int32)
    W1, b1, W2, b2, W3, b3, Wp, bp = (np.asarray(a, np.float32)
                                      for a in (W1, b1, W2, b2, W3, b3, Wp, bp))
    pre, nc = _get_program(edge_index)

    w_all = np.concatenate([W1, W2, W3], axis=1)              # [128, 384]
    b_all = np.concatenate([b1, b2, b3]).reshape(1, 3 * D)
    wp_all = np.concatenate([Wp[:D], Wp[D:2 * D], Wp[2 * D:]], axis=1)  # [128,30]
    iota = np.broadcast_to(np.arange(128, dtype=np.float32), (128, 128)).copy()
    ident = np.eye(128, dtype=np.float32)

    feat_p = np.zeros((NCORES, SHARD_P, D), np.float32)
    feat_p[:, :SHARD] = feat.reshape(NCORES, SHARD, D)

    in_maps = []
    for c in range(NCORES):
        in_maps.append({
            "feat": feat_p[c],
            "idx": pre["idx"][c],
            "tgt": pre["tgt"][c],
            "deg_col": pre["deg_col"][c],
            "deg_row": pre["deg_row"][c],
            "w_all": w_all, "b_all": b_all,
            "wp_all": wp_all, "bp": bp.reshape(1, D_LAB),
            "iota": iota, "ident": ident,
        })
    return nc, in_maps


def kernel(**inputs):
    nc, in_maps = prepare(**inputs)
    trace = bool(int(os.environ.get("GCN_TRACE", "0")))
    res = bass_utils.run_bass_kernel_spmd(nc, in_maps,
                                          core_ids=list(range(NCORES)),
                                          trace=trace)
    global LAST_RESULTS
    LAST_RESULTS = res
    out = np.empty((N_NODES, D_LAB), np.float32)
    for c in range(NCORES):
        out[c * SHARD:(c + 1) * SHARD] = res.results[c]["out"].T[:SHARD]
    return out


LAST_RESULTS = None


# revision 15
# speedup vs baseline: 423.7880x; 2.1258x over previous
"""GCN (3-layer + linear head) Trainium2 Bass kernel, sharded over 8 NeuronCores.

Strategy (matches the vertex-partitioning sharding hint):
 - Nodes are sharded contiguously: core c owns nodes [c*12500, (c+1)*12500),
   padded to 12544 = 98 blocks of 128 rows.
 - Per GCN layer, each core transforms its local rows (y = dinv * (h @ W)),
   the y shards are AllGathered (boundary/halo exchange), and each core
   aggregates messages for its own targets with dma_gather (per-edge source
   row fetch) + one-hot matmuls on the tensor engine that realize the
   segment-sum in PSUM.
 - All floating-point math runs on device. The host only does integer
   index work: adding self-loops, degree counting (bincount), sorting edges
   by (target-block-group, source-quarter, target), padding chunk counts so
   all 8 cores share one SPMD instruction stream.
"""
import os
import sys

sys.path.insert(0, "/opt/trn_rl_repo")

import numpy as np

_NLAYERS = int(os.environ.get("GCN_NLAYERS", "3"))
_SKIP_AGG = bool(int(os.environ.get("GCN_SKIP_AGG", "0")))
_SKIP_FINAL = bool(int(os.environ.get("GCN_SKIP_FINAL", "0")))
_MAX_GROUPS = int(os.environ.get("GCN_MAX_GROUPS", "999"))
_NO_GATHER = bool(int(os.environ.get("GCN_NO_GATHER", "0")))
_NO_SBUILD = bool(int(os.environ.get("GCN_NO_SBUILD", "0")))
_NO_MM = bool(int(os.environ.get("GCN_NO_MM", "0")))
_ONECORE = bool(int(os.environ.get("GCN_ONECORE", "0")))

import concourse.bacc as bacc
import concourse.mybir as mybir
import concourse.tile as tile
from concourse import bass_utils
from concourse.library_config import mlp

# Problem constants (hardcoded per harness contract).
N_NODES = 100000
D = 128
D_LAB = 10
NCORES = 8
SHARD = 12500
SHARD_P = 12544           # padded to 98 * 128
B = SHARD_P // 128        # 98 blocks per core
G = 6                     # target blocks per aggregation group (psum banks)
NG = -(-B // G)           # 17 groups
QROWS = 32768             # int16 index range per gather quarter
NQ = -(-(NCORES * SHARD_P) // QROWS)  # 4 quarters over padded global rows

F32 = mybir.dt.float32
I16 = mybir.dt.int16
AF = mybir.ActivationFunctionType
ALU = mybir.AluOpType

PAD_TGT = 9999.0          # one-hot target value for padded slots (matches nothing)


def _preprocess(edge_index):
    """Host-side integer/index prep. Returns per-core arrays + shared structure."""
    src = np.asarray(edge_index[0], dtype=np.int64)
    tgt = np.asarray(edge_index[1], dtype=np.int64)
    loops = np.arange(N_NODES, dtype=np.int64)
    src = np.concatenate([src, loops])
    tgt = np.concatenate([tgt, loops])

    deg = np.bincount(tgt, minlength=N_NODES).astype(np.int64)  # includes self loop

    core = tgt // SHARD
    tl = tgt % SHARD                       # target local to core
    gs = (src // SHARD) * SHARD_P + (src % SHARD)  # padded global source row
    q = gs // QROWS
    qrel = (gs - q * QROWS).astype(np.int64)
    blk = tl // 128
    grp = blk // G

    order = np.lexsort((tl, q, grp, core))
    core_s, tl_s, q_s, qrel_s, blk_s = (
        core[order], tl[order], q[order], qrel[order], blk[order])

    # segment = (core, g, q, blk); count edges per segment
    seg_key = ((core_s * NG + (blk_s // G)) * NQ + q_s) * B + blk_s
    counts = np.bincount(seg_key, minlength=NCORES * NG * NQ * B).reshape(
        NCORES, NG, NQ, B)
    # structural chunk count per (g, q, blk): max over cores
    C = -(-counts // 128)
    C = C.max(axis=0)  # [NG, NQ, B]

    # Build flat idx / tgt arrays per core, in (g, q, b, chunk) order.
    tot_chunks = int(C.sum())
    TOT = tot_chunks * 128
    idx_all = np.zeros((NCORES, TOT), dtype=np.int16)
    tgt_all = np.full((NCORES, tot_chunks * 128), PAD_TGT, dtype=np.float32)

    # per-core start offset of each segment in the sorted arrays
    seg_starts = np.zeros(NCORES * NG * NQ * B + 1, dtype=np.int64)
    np.cumsum(np.bincount(seg_key, minlength=NCORES * NG * NQ * B),
              out=seg_starts[1:])

    # shared structure for the builder
    segs = []       # list of (g, q, b, n_chunks) in emission order
    nch_gq = np.zeros((NG, NQ), dtype=np.int64)
    off = 0
    for g in range(NG):
        for qq in range(NQ):
            for b in range(g * G, min((g + 1) * G, B)):
                nch = int(C[g, qq, b])
                if nch == 0:
                    continue
                segs.append((g, qq, b, nch, off))
                nch_gq[g, qq] += nch
                for c in range(NCORES):
                    k = ((c * NG + g) * NQ + qq) * B + b
                    s0, s1 = seg_starts[k], seg_starts[k + 1]
                    n = s1 - s0
                    idx_all[c, off:off + n] = qrel_s[s0:s1]
                    tgt_all[c, off:off + n] = (tl_s[s0:s1] - b * 128)
                off += nch * 128
    assert off == TOT

    # wrap idxs to [128, TOT/16]: idx i -> [i % 16, i // 16], tiled x8
    idx_wrapped = np.stack([
        np.tile(a.reshape(-1, 16).T, (8, 1)) for a in idx_all])
    # tgt values in [128, tot_chunks]: chunk k, slot p -> [p, k]
    tgt_tiles = tgt_all.reshape(NCORES, tot_chunks, 128).transpose(0, 2, 1).copy()

    # degrees, padded shards; pad deg with 1 to avoid inf (padded rows harmless)
    deg_p = np.ones((NCORES, SHARD_P), dtype=np.float32)
    deg_p[:, :SHARD] = deg.reshape(NCORES, SHARD).astype(np.float32)
    deg_col = deg_p.reshape(NCORES, B, 128).transpose(0, 2, 1).copy()  # [c,128,B]
    deg_row = deg_p.reshape(NCORES, 1, SHARD_P)

    return dict(idx=idx_wrapped, tgt=tgt_tiles, deg_col=deg_col, deg_row=deg_row,
                segs=segs, nch_gq=nch_gq, tot_chunks=tot_chunks, TOT=TOT)


def _build(pre):
    """Build the Bass/Tile program (one SPMD NEFF for all 8 cores)."""
    TOT = pre["TOT"]
    tot_chunks = pre["tot_chunks"]
    nch_gq = pre["nch_gq"]
    segs = pre["segs"]

    nc = bacc.Bacc("TRN2", target_bir_lowering=False, debug=False,
                   num_devices=1 if _ONECORE else NCORES)

    feat_d = nc.dram_tensor("feat", [SHARD_P, D], F32, kind="ExternalInput")
    idx_d = nc.dram_tensor("idx", [128, TOT // 16], I16, kind="ExternalInput")
    tgt_d = nc.dram_tensor("tgt", [128, tot_chunks], F32, kind="ExternalInput")
    degc_d = nc.dram_tensor("deg_col", [128, B], F32, kind="ExternalInput")
    degr_d = nc.dram_tensor("deg_row", [1, SHARD_P], F32, kind="ExternalInput")
    w_d = nc.dram_tensor("w_all", [128, 3 * D], F32, kind="ExternalInput")
    b_d = nc.dram_tensor("b_all", [1, 3 * D], F32, kind="ExternalInput")
    wp_d = nc.dram_tensor("wp_all", [128, 3 * D_LAB], F32, kind="ExternalInput")
    bp_d = nc.dram_tensor("bp", [1, D_LAB], F32, kind="ExternalInput")
    iota_d = nc.dram_tensor("iota", [128, 128], F32, kind="ExternalInput")
    ident_d = nc.dram_tensor("ident", [128, 128], F32, kind="ExternalInput")

    out_d = nc.dram_tensor("out", [D_LAB, SHARD_P], F32, kind="ExternalOutput")

    with tile.TileContext(nc) as tc:
        with (
            tc.tile_pool(name="const", bufs=1) as cpool,
            tc.tile_pool(name="work", bufs=3) as wpool,
            tc.tile_pool(name="sbuild", bufs=8) as spool,
            tc.tile_pool(name="mtiles", bufs=2) as mpool,
            tc.tile_pool(name="psum_a", bufs=G, space="PSUM") as ppa,
            tc.tile_pool(name="psum_t", bufs=2, space="PSUM") as ppt,
            tc.tile_pool(name="dram", bufs=1, space="DRAM") as dpool,
        ):
            nc.gpsimd.load_library(mlp)

            # ---- constants ----
            tgt_s = cpool.tile([128, tot_chunks], F32)
            iota_s = cpool.tile([128, 128], F32)
            ident_s = cpool.tile([128, 128], F32)
            w_s = cpool.tile([128, 3 * D], F32)
            b_s = cpool.tile([1, 3 * D], F32)
            wp_s = cpool.tile([128, 3 * D_LAB], F32)
            bp_s = cpool.tile([1, D_LAB], F32)
            ones_s = cpool.tile([1, 128], F32)
            degc_s = cpool.tile([128, B], F32)
            recip_s = cpool.tile([128, B], F32)
            dinv_s = cpool.tile([128, B], F32)
            sqdeg_s = cpool.tile([1, SHARD_P], F32)

            nc.sync.dma_start(tgt_s[:], tgt_d[:])
            nc.sync.dma_start(iota_s[:], iota_d[:])
            nc.sync.dma_start(ident_s[:], ident_d[:])
            nc.sync.dma_start(w_s[:], w_d[:])
            nc.sync.dma_start(b_s[:], b_d[:])
            nc.sync.dma_start(wp_s[:], wp_d[:])
            nc.sync.dma_start(bp_s[:], bp_d[:])
            nc.sync.dma_start(degc_s[:], degc_d[:])
            nc.sync.dma_start(sqdeg_s[:], degr_d[:])
            nc.vector.memset(ones_s[:], 1.0)

            # dinv = sqrt(1/deg)  (accurate DVE reciprocal + ACT sqrt)
            nc.vector.reciprocal(recip_s[:], degc_s[:])
            nc.scalar.sqrt(dinv_s[:], recip_s[:])
            nc.scalar.sqrt(sqdeg_s[:], sqdeg_s[:])  # in-place: deg -> sqrt(deg)

            # ---- internal DRAM ----
            y_loc = dpool.tile([SHARD_P, D], F32)
            y_fulls = [
                dpool.tile([NCORES * SHARD_P, D], F32,
                           addr_space="Local" if _ONECORE else "Shared",
                           name=f"y_full_{i}")
                for i in range(3)
            ]
            h1 = dpool.tile([SHARD_P, D], F32)
            h2 = dpool.tile([SHARD_P, D], F32)
            h3 = dpool.tile([SHARD_P, D], F32)

            h_tensors = [feat_d, h1, h2, h3]

            for layer in range(_NLAYERS):
                hin = h_tensors[layer]
                hout = h_tensors[layer + 1]
                wl = w_s[:, layer * D:(layer + 1) * D]
                bl = b_s[:, layer * D:(layer + 1) * D]

                # -- transform: y = dinv * (hin @ W) --
                for b in range(B):
                    r0, r1 = b * 128, (b + 1) * 128
                    hb = wpool.tile([128, 128], F32, tag="hb")
                    nc.sync.dma_start(hb[:], hin[r0:r1, :])
                    tp = ppt.tile([128, 128], F32, tag="tp")
                    nc.tensor.transpose(tp[:], hb[:], ident_s[:])
                    hT = wpool.tile([128, 128], F32, tag="hT")
                    nc.vector.tensor_copy(hT[:], tp[:])
                    yp = ppt.tile([128, 128], F32, tag="tp")
                    nc.tensor.matmul(yp[:], hT[:], wl, start=True, stop=True)
                    yb = wpool.tile([128, 128], F32, tag="yb")
                    nc.scalar.activation(yb[:], yp[:], AF.Copy,
                                         bias=0.0, scale=dinv_s[:, b:b + 1])
                    nc.sync.dma_start(y_loc[r0:r1, :], yb[:])

                # -- halo exchange --
                y_full = y_fulls[layer]
                if _ONECORE:
                    nc.sync.dma_start(y_full[0:SHARD_P, :], y_loc[:])
                else:
                    nc.gpsimd.collective_compute(
                        "AllGather", ALU.bypass,
                        replica_groups=[list(range(NCORES))],
                        ins=[y_loc.opt()], outs=[y_full.opt()],
                    )

                # -- aggregate --
                if _SKIP_AGG:
                    continue
                seg_i = 0
                ci = 0          # global chunk counter (matches tgt_s columns)
                off16 = 0       # idx column offset
                for g in range(min(NG, _MAX_GROUPS)):
                    blocks = list(range(g * G, min((g + 1) * G, B)))
                    psums = {b: ppa.tile([128, 128], F32, tag="agg",
                                         name=f"ps_{layer}_{b}")
                             for b in blocks}
                    started = {b: False for b in blocks}
                    for qq in range(NQ):
                        nch = int(nch_gq[g, qq])
                        if nch == 0:
                            continue
                        n_idx = nch * 128
                        n16 = n_idx // 16
                        idx_t = mpool.tile([128, n16], I16, tag="idxs",
                                           name=f"ix_{layer}_{g}_{qq}")
                        nc.sync.dma_start(idx_t[:],
                                          idx_d[:, off16:off16 + n16])
                        mt = mpool.tile([128, nch, 128], F32, tag="m",
                                        name=f"m_{layer}_{g}_{qq}")
                        qs = qq * QROWS
                        qe = min(qs + QROWS, NCORES * SHARD_P)
                        if not _NO_GATHER:
                            nc.gpsimd.dma_gather(
                                mt[:], y_full[qs:qe, :], idx_t[:],
                                n_idx, n_idx, D, single_packet=False)
                        off16 += n16
                        k = 0
                        while k < nch:
                            g2, q2, b2, nck, _ = segs[seg_i]
                            assert g2 == g and q2 == qq
                            for _ in range(nck):
                                st = spool.tile([128, 128], F32, tag="s",
                                                name=f"s_{layer}_{ci}")
                                if not _NO_SBUILD:
                                    nc.vector.tensor_scalar(
                                        st[:], iota_s[:],
                                        tgt_s[:, ci:ci + 1], None,
                                        ALU.is_equal)
                                if not _NO_MM:
                                    nc.tensor.matmul(
                                        psums[b2][:], st[:], mt[:, k, :],
                                        start=not started[b2], stop=False)
                                started[b2] = True
                                k += 1
                                ci += 1
                            seg_i += 1
                    for b in blocks:
                        assert started[b]
                        r0, r1 = b * 128, (b + 1) * 128
                        # psum += sqrt(deg) x bias  (rank-1)
                        nc.tensor.matmul(psums[b][:], sqdeg_s[:, r0:r1], bl,
                                         start=False, stop=True)
                        ob = wpool.tile([128, 128], F32, tag="ob")
                        func = AF.Relu if layer < 2 else AF.Copy
                        nc.scalar.activation(ob[:], psums[b][:], func,
                                             bias=0.0, scale=dinv_s[:, b:b + 1])
                        nc.sync.dma_start(hout[r0:r1, :], ob[:])
                if _MAX_GROUPS >= NG:
                    assert seg_i == len(segs) and ci == tot_chunks

            # -- final projection: out[l, t] = sum_i h_i @ Wp_i + bp --
            for b in range(B if not (_SKIP_FINAL or _NLAYERS < 3) else 0):
                r0, r1 = b * 128, (b + 1) * 128
                pf = ppt.tile([D_LAB, 128], F32, tag="tp", name=f"pf_{b}")
                for i, hd in enumerate((h1, h2, h3)):
                    fb = wpool.tile([128, 128], F32, tag="hb")
                    nc.sync.dma_start(fb[:], hd[r0:r1, :])
                    ftp = ppt.tile([128, 128], F32, tag="tp")
                    nc.tensor.transpose(ftp[:], fb[:], ident_s[:])
                    fT = wpool.tile([128, 128], F32, tag="hT")
                    nc.vector.tensor_copy(fT[:], ftp[:])
                    nc.tensor.matmul(pf[:], wp_s[:, i * D_LAB:(i + 1) * D_LAB],
                                     fT[:], start=(i == 0), stop=False)
                nc.tensor.matmul(pf[:], bp_s[:], ones_s[:],
                                 start=False, stop=True)
                fo = wpool.tile([D_LAB, 128], F32, tag="fo")
                nc.scalar.activation(fo[:], pf[:], AF.Copy)
                nc.sync.dma_start(out_d[:, r0:r1], fo[:])

    nc.compile()
    return nc


_CACHE = {}


def _get_program(edge_index):
    key = hash(np.asarray(edge_index).tobytes())
    if key not in _CACHE:
        pre = _preprocess(edge_index)
        nc = _build(pre)
        _CACHE.clear()
        _CACHE[key] = (pre, nc)
    return _CACHE[key]


def prepare(feat, edge_index, W1, b1, W2, b2, W3, b3, Wp, bp):
    """Build (nc, in_maps) for the SPMD run."""
    feat = np.asarray(feat, np.float32)
    edge_index = np.asarray(edge_index, np.int32)
    W1, b1, W2, b2, W3, b3, Wp, bp = (np.asarray(a, np.float32)
                                      for a in (W1, b1, W2, b2, W3, b3, Wp, bp))
    pre, nc = _get_program(edge_index)

    w_all = np.concatenate([W1, W2, W3], axis=1)              # [128, 384]
    b_all = np.concatenate([b1, b2, b3]).reshape(1, 3 * D)
    wp_all = np.concatenate([Wp[:D], Wp[D:2 * D], Wp[2 * D:]], axis=1)  # [128,30]
    iota = np.broadcast_to(np.arange(128, dtype=np.float32), (128, 128)).copy()
    ident = np.eye(128, dtype=np.float32)

    feat_p = np.zeros((NCORES, SHARD_P, D), np.float32)
    feat_p[:, :SHARD] = feat.reshape(NCORES, SHARD, D)

    in_maps = []
    for c in range(NCORES):
        in_maps.append({
            "feat": feat_p[c],
            "idx": pre["idx"][c],
            "tgt": pre["tgt"][c],
            "deg_col": pre["deg_col"][c],
            "deg_row": pre["deg_row"][c],
            "w_all": w_all, "b_all": b_all,
            "wp_all": wp_all, "bp": bp.reshape(1, D_LAB),
            "iota": iota, "ident": ident,
        })
    return nc, in_maps


def kernel(**inputs):
    nc, in_maps = prepare(**inputs)
    trace = bool(int(os.environ.get("GCN_TRACE", "0")))
    res = bass_utils.run_bass_kernel_spmd(nc, in_maps,
                                          core_ids=list(range(NCORES)),
                                          trace=trace)
    global LAST_RESULTS
    LAST_RESULTS = res
    out = np.empty((N_NODES, D_LAB), np.float32)
    for c in range(NCORES):
        out[c * SHARD:(c + 1) * SHARD] = res.results[c]["out"].T[:SHARD]
    return out


LAST_RESULTS = None
